# revision 31
# baseline (speedup 1.0000x reference)
"""Performer (FAVOR+) encoder layer on 8 trn2 NeuronCores.

Sharding: data-parallel over sequence (512 positions per core x 4 batches).
The linear-attention summaries (A = E_k^T v per (batch, head), usum) and the
global key-feature max (via one-hot slots) are combined in ONE packed
AllReduce, overlapped with the Q-side feature compute.

Layout: activations feature-major (xT = [D, tokens]) so every GEMM's
stationary operand is a natural weight slice; per-token reductions and
broadcasts are small PE matmuls (selector / ones / f32r broadcast matmuls).
E_k and v are produced token-major directly by matmuls so the token-
contraction A-matmul needs no transposes.
"""
import os
import numpy as np
import ml_dtypes

B, N, D = 4, 4096, 1024
H, DH = 16, 64
DFF = 4096
M = 64
EPS_KERN = 1e-6
EPS_LN = 1e-6
NC = 8
NT = N // NC                # 512 positions per core per batch
PAIRS = H // 2              # 8 head-pairs
KT_D = D // 128             # 8
MT_FF = DFF // 128          # 32
TT = NT // 128              # 4
DN = 1.0 / np.sqrt(np.sqrt(DH))
DN2H = DN * DN / 2.0


def _emit(nc, tc):
    import concourse.mybir as mybir
    from contextlib import ExitStack
    F32 = mybir.dt.float32
    F32R = mybir.dt.float32r
    BF16 = mybir.dt.bfloat16
    AF = mybir.ActivationFunctionType
    ALU = mybir.AluOpType
    AX = mybir.AxisListType

    dram = lambda name, shape, dt, kind: nc.dram_tensor(name, shape, dt, kind=kind).ap()

    x_bf = dram("x_bf", [B, D, NT], BF16, "ExternalInput")
    x_f32 = dram("x_f32", [B, D, NT], F32, "ExternalInput")
    wqs = dram("wqs", [KT_D, 128, KT_D, 128], BF16, "ExternalInput")
    wks = dram("wks", [KT_D, 128, KT_D, 128], BF16, "ExternalInput")
    wv = dram("wv", [D, D], BF16, "ExternalInput")
    wos = dram("wos", [KT_D, 128, KT_D, 128], BF16, "ExternalInput")
    w1s = dram("w1s", [MT_FF, 128, KT_D, 128], BF16, "ExternalInput")
    w2s = dram("w2s", [KT_D, 128, MT_FF, 128], BF16, "ExternalInput")
    projbd = dram("projbd", [128, 128], BF16, "ExternalInput")
    negselF = dram("negselF", [2, 128], BF16, "ExternalInput")
    sel2 = dram("sel2", [128, 2], BF16, "ExternalInput")
    sel2b = dram("sel2b", [2, 128], F32, "ExternalInput")
    ones128 = dram("ones128", [128, 1], BF16, "ExternalInput")
    ones1x128 = dram("ones1x128", [1, 128], F32, "ExternalInput")
    mean1 = dram("mean1", [128, 1], F32, "ExternalInput")
    headmask2 = dram("headmask2", [128, 2], F32, "ExternalInput")
    epsvA = dram("epsvA", [128, B * PAIRS, 64], BF16, "ExternalInput")
    onehot = dram("onehot", [1, NC], F32, "ExternalInput")
    b1c = dram("b1c", [128, MT_FF], F32, "ExternalInput")
    b1p1c = dram("b1p1c", [128, MT_FF], F32, "ExternalInput")
    b2adjc = dram("b2adjc", [128, KT_D], F32, "ExternalInput")
    g1c = dram("g1c", [128, KT_D], F32, "ExternalInput")
    be1c = dram("be1c", [128, KT_D], F32, "ExternalInput")
    g2c = dram("g2c", [128, KT_D], F32, "ExternalInput")
    be2c = dram("be2c", [128, KT_D], F32, "ExternalInput")
    out = dram("out", [B, D, NT], F32, "ExternalOutput")

    AC_A = B * PAIRS * 64
    AC_U = B * PAIRS
    AC = AC_A + AC_U + NC

    ctx = ExitStack()
    pconst = ctx.enter_context(tc.tile_pool(name="pconst", bufs=1))
    pwv = ctx.enter_context(tc.tile_pool(name="pwv", bufs=1))
    pstream = ctx.enter_context(tc.tile_pool(name="pstream", bufs=2))
    pw2s = ctx.enter_context(tc.tile_pool(name="pw2s", bufs=2))
    pxa = ctx.enter_context(tc.tile_pool(name="pxa", bufs=1))
    pxb = ctx.enter_context(tc.tile_pool(name="pxb", bufs=2))
    pmt = ctx.enter_context(tc.tile_pool(name="pmt", bufs=3))
    psm = ctx.enter_context(tc.tile_pool(name="psm", bufs=2))
    pln = ctx.enter_context(tc.tile_pool(name="pln", bufs=1))
    peq = ctx.enter_context(tc.tile_pool(name="peq", bufs=2))
    pbig = ctx.enter_context(tc.tile_pool(name="pbig", bufs=1))
    pbig2 = ctx.enter_context(tc.tile_pool(name="pbig2", bufs=2))
    pone = ctx.enter_context(tc.tile_pool(name="pone", bufs=1))
    pdram = ctx.enter_context(tc.tile_pool(name="pdram", bufs=1, space="DRAM"))
    PP = ctx.enter_context(tc.tile_pool(name="PP", bufs=4, space="PSUM"))
    PD = ctx.enter_context(tc.tile_pool(name="PD", bufs=2, space="PSUM"))
    PA_ = ctx.enter_context(tc.tile_pool(name="PA", bufs=1, space="PSUM"))
    PR = ctx.enter_context(tc.tile_pool(name="PR", bufs=1, space="PSUM"))

    # ---- constants ----
    wv_sb = pwv.tile([128, KT_D, D], BF16, tag="wv")
    nc.sync.dma_start(wv_sb[:], wv.rearrange("(kt p) m -> p kt m", p=128))
    cAPs = {}
    for name, ap, shape, dt in (
        ("projbd", projbd, [128, 128], BF16), ("negselF", negselF, [2, 128], BF16),
        ("sel2", sel2, [128, 2], BF16), ("sel2b", sel2b, [2, 128], F32),
        ("ones128", ones128, [128, 1], BF16), ("ones1x128", ones1x128, [1, 128], F32),
        ("mean1", mean1, [128, 1], F32), ("headmask2", headmask2, [128, 2], F32),
        ("onehot", onehot, [1, NC], F32), ("b1c", b1c, [128, MT_FF], F32),
        ("b1p1c", b1p1c, [128, MT_FF], F32), ("b2adjc", b2adjc, [128, KT_D], F32),
        ("g1c", g1c, [128, KT_D], F32), ("be1c", be1c, [128, KT_D], F32),
        ("g2c", g2c, [128, KT_D], F32), ("be2c", be2c, [128, KT_D], F32),
    ):
        t = pconst.tile(shape, dt, tag=name)
        nc.sync.dma_start(t[:], ap[:])
        cAPs[name] = t
    sel2b_r = pconst.tile([2, 128], F32R, tag="sel2br")
    ones1x128_r = pconst.tile([1, 128], F32R, tag="ones1x128r")
    mean1_r = pconst.tile([128, 1], F32R, tag="mean1r")
    mean1_bf = pconst.tile([128, 1], BF16, tag="mean1bf")
    sel2b_bf = pconst.tile([2, 128], BF16, tag="sel2bbf")
    nc.vector.tensor_copy(sel2b_r[:], cAPs["sel2b"][:])
    nc.vector.tensor_copy(ones1x128_r[:], cAPs["ones1x128"][:])
    nc.vector.tensor_copy(mean1_r[:], cAPs["mean1"][:])
    nc.vector.tensor_copy(mean1_bf[:], cAPs["mean1"][:])
    nc.vector.tensor_copy(sel2b_bf[:], cAPs["sel2b"][:])

    epsln_c = pconst.tile([1, 1], F32, tag="epslnc")
    nc.vector.memset(epsln_c[:], float(EPS_LN))
    onesrow_c = pconst.tile([1, NT], F32, tag="onesrowc")
    nc.vector.memset(onesrow_c[:], 1.0)
    ones1x128_bf = pconst.tile([1, 128], BF16, tag="ones1x128bf")
    nc.vector.tensor_copy(ones1x128_bf[:], cAPs["ones1x128"][:])
    arstA = pone.tile([128, AC_A], BF16, tag="arbufA")
    arstS = pone.tile([128, AC_U + NC], F32, tag="arbufS")
    mxcols = pone.tile([128, B * PAIRS * TT], BF16, tag="mxcols")

    def ln(res, gc, bc, tag, odt, dma_out=None):
        """Feature-major LN over a bf16 [128, KT_D, NT] residual tile.
        Stats via bf16 PE reductions; mu/rstd broadcast to all partitions
        via K=1 matmuls, evacuated to bf16 so the per-kt chain runs at
        DVE 2x rate."""
        pm0 = PR.tile([1, NT], F32, tag="prow")
        pm1 = PA_.tile([1, NT], F32, tag="pA")
        for kt in range(KT_D):
            sqt = pln.tile([128, NT], BF16, tag="lnsq")
            nc.scalar.square(sqt[:], res[:, kt, :])
            nc.tensor.matmul(pm0[:], mean1_bf[:], res[:, kt, :],
                             start=kt == 0, stop=kt == KT_D - 1,
                             skip_group_check=True)
            nc.tensor.matmul(pm1[:], mean1_bf[:], sqt[:],
                             start=kt == 0, stop=kt == KT_D - 1,
                             skip_group_check=True)
        mu = pln.tile([1, NT], BF16, tag="lnmu")
        nc.vector.tensor_copy(mu[:], pm0[:])
        mu2 = pln.tile([1, NT], F32, tag="lnrowA")
        nc.vector.tensor_tensor(mu2[:], mu[:], mu[:], op=ALU.mult)
        var = pln.tile([1, NT], F32, tag="lnrowB")
        nc.vector.tensor_tensor(var[:], pm1[:], mu2[:], op=ALU.subtract)
        lnv = pln.tile([1, NT], F32, tag="lnrowA")
        nc.scalar.activation(lnv[:], var[:], AF.Ln, bias=epsln_c[:])
        rstdf = pln.tile([1, NT], BF16, tag="lnrowB2")
        nc.scalar.activation(rstdf[:], lnv[:], AF.Exp, scale=-0.5)
        pmu = PP.tile([128, NT], F32, tag="pbig")
        nc.tensor.matmul(pmu[:], ones1x128_bf[:], mu[:], start=True, stop=True)
        prs = PP.tile([128, NT], F32, tag="pbig")
        nc.tensor.matmul(prs[:], ones1x128_bf[:], rstdf[:], start=True, stop=True)
        muB = pln.tile([128, NT], BF16, tag="lnmuB")
        nc.vector.tensor_copy(muB[:], pmu[:])
        rsB = pln.tile([128, NT], BF16, tag="lnrsB")
        nc.vector.tensor_copy(rsB[:], prs[:])
        o = None
        if odt is not None:
            o = pbig2.tile([128, KT_D, NT], odt, tag="bf8")
        cen = pln.tile([128, NT], BF16, tag="lncen")
        nrm = pln.tile([128, NT], BF16, tag="lnnrm")
        for kt in range(KT_D):
            nc.vector.tensor_tensor(cen[:], res[:, kt, :], muB[:],
                                    op=ALU.subtract)
            nc.vector.tensor_tensor(nrm[:], cen[:], rsB[:], op=ALU.mult)
            if o is not None:
                nc.vector.tensor_scalar(o[:, kt, :], nrm[:], gc[:, kt:kt + 1],
                                        bc[:, kt:kt + 1], op0=ALU.mult, op1=ALU.add)
            else:
                ot = pln.tile([128, NT], F32, tag="lnot")
                nc.vector.tensor_scalar(ot[:], nrm[:], gc[:, kt:kt + 1],
                                        bc[:, kt:kt + 1], op0=ALU.mult, op1=ALU.add)
                nc.sync.dma_start(dma_out[0][dma_out[1], kt * 128:(kt + 1) * 128, :],
                                  ot[:])
        return o

    # ================= stage A =================
    vtok = pxa.tile([128, TT, PAIRS, 129], BF16, tag="vtok")
    nc.vector.memset(vtok[:], 1.0)
    NB_A = PAIRS * 64           # AR payload columns per batch

    def fire_ar(tag, src_ap, ncols, dt=None):
        dt = dt or F32
        ain = pdram.tile([128, ncols], dt, tag=f"arin{tag}")
        aout = pdram.tile([128, ncols], dt, tag=f"arout{tag}",
                          addr_space="Shared")
        nc.sync.dma_start(ain[:], src_ap)
        if os.environ.get("KERNEL_NOCOLL"):
            nc.sync.dma_start(aout[:], ain[:])
        else:
            nc.gpsimd.collective_compute("AllReduce", ALU.add,
                                         replica_groups=[list(range(NC))],
                                         ins=[ain[:]], outs=[aout[:]])
        return aout

    ar_outs = {}
    for b in range(B):
        xbf = pxb.tile([128, KT_D, NT], BF16, tag="xbf")
        nc.sync.dma_start(xbf[:], x_bf[b].rearrange("(kt p) t -> p kt t", p=128))

        for tt in range(TT):
            for nh in range(2):
                pv = PP.tile([128, NT], F32, tag="pbig")
                for kt in range(KT_D):
                    nc.tensor.matmul(pv[:], xbf[:, kt, tt * 128:(tt + 1) * 128],
                                     wv_sb[:, kt, nh * 512:(nh + 1) * 512],
                                     start=kt == 0, stop=kt == KT_D - 1)
                for i in range(4):
                    nc.scalar.activation(vtok[:, tt, nh * 4 + i, 0:128],
                                         pv[:, i * 128:(i + 1) * 128], AF.Copy)

        kq = {}

        def kproj(pr):
            wkmt = pstream.tile([128, KT_D, 128], BF16, tag="wmt")
            nc.sync.dma_start(wkmt[:], wks[pr])
            pk = PP.tile([128, NT], F32, tag="pbig")
            for kt in range(KT_D):
                nc.tensor.matmul(pk[:], wkmt[:, kt, :], xbf[:, kt, :],
                                 start=kt == 0, stop=kt == KT_D - 1)
            kTmt = pmt.tile([128, NT], BF16, tag="mt512")
            nc.scalar.activation(kTmt[:], pk[:], AF.Copy)
            ksqmt = pmt.tile([128, NT], BF16, tag="mt512")
            nc.scalar.square(ksqmt[:], pk[:])
            kq[pr] = (kTmt, ksqmt)

        def ksmall(pr):
            kTmt, ksqmt = kq[pr]
            pks = PR.tile([2, NT], F32, tag="prow")
            nc.tensor.matmul(pks[:], cAPs["sel2"][:], ksqmt[:], start=True,
                             stop=True)
            ksq2 = psm.tile([2, NT], BF16, tag="ksq2")
            nc.scalar.activation(ksq2[:], pks[:], AF.Copy)

            Ek = psm.tile([128, TT, 128], BF16, tag="Ek")
            for tt in range(TT):
                pdd = PD.tile([128, 128], F32, tag="pdd")
                nc.tensor.matmul(pdd[:], kTmt[:, tt * 128:(tt + 1) * 128],
                                 cAPs["projbd"][:], start=True, stop=True)
                c = (b * PAIRS + pr) * TT + tt
                nc.vector.tensor_reduce(mxcols[:, c:c + 1], pdd[:], axis=AX.X,
                                        op=ALU.max)
                nc.tensor.matmul(pdd[:], ksq2[:, tt * 128:(tt + 1) * 128],
                                 cAPs["negselF"][:], start=False, stop=True,
                                 skip_group_check=True)
                nc.scalar.activation(Ek[:, tt, :], pdd[:], AF.Exp)

            pA = PA_.tile([128, 129], F32, tag="pA")
            for tt in range(TT):
                nc.tensor.matmul(pA[:], Ek[:, tt, :], vtok[:, tt, pr, :],
                                 start=tt == 0, stop=tt == TT - 1,
                                 skip_group_check=True)
            j = b * PAIRS + pr
            nc.vector.tensor_copy(arstA[0:64, j * 64:(j + 1) * 64],
                                  pA[0:64, 0:64])
            nc.vector.tensor_copy(arstA[64:128, j * 64:(j + 1) * 64],
                                  pA[64:128, 64:128])
            nc.vector.tensor_copy(arstS[:, j:j + 1], pA[:, 128:129])

        kproj(0)
        for pr in range(PAIRS):
            if pr + 1 < PAIRS:
                kproj(pr + 1)
            ksmall(pr)

        ar_outs[b] = fire_ar(f"A{b}", arstA[:, b * NB_A:(b + 1) * NB_A],
                             NB_A, BF16)

    # ---- fire final (usum + max) AllReduce ----
    mxr = pone.tile([128, 1], F32, tag="mxr")
    nc.vector.tensor_reduce(mxr[:], mxcols[:], axis=AX.X, op=ALU.max)
    mx1 = pone.tile([1, 1], F32, tag="mx1")
    nc.gpsimd.tensor_reduce(mx1[:], mxr[:], axis=AX.C, op=ALU.max)
    nc.vector.tensor_scalar(arstS[0:1, AC_U:AC_U + NC], cAPs["onehot"][:],
                            mx1[:], None, op0=ALU.mult)
    ar_small = fire_ar("S", arstS[:], AC_U + NC)

    arresA = pone.tile([128, AC_A], BF16, tag="arbufA")
    arresS = pone.tile([128, AC_U + NC], F32, tag="arbufS")
    for b in range(B):
        nc.sync.dma_start(arresA[:, b * NB_A:(b + 1) * NB_A], ar_outs[b][:])
    nc.sync.dma_start(arresS[:], ar_small[:])

    # ================= q-side features (overlap AR) =================
    Eq_all = {}

    def emit_qside(b, fold=False):
        qxbf = pxa.tile([128, KT_D, NT], BF16, tag="qxbf")
        nc.sync.dma_start(qxbf[:], x_bf[b].rearrange("(kt p) t -> p kt t", p=128))
        Eqs = []
        qq = {}

        def qproj(pr):
            wqmt = pstream.tile([128, KT_D, 128], BF16, tag="wmt")
            nc.sync.dma_start(wqmt[:], wqs[pr])
            pq_ = PP.tile([128, NT], F32, tag="pbig")
            for kt in range(KT_D):
                nc.tensor.matmul(pq_[:], wqmt[:, kt, :], qxbf[:, kt, :],
                                 start=kt == 0, stop=kt == KT_D - 1)
            qTmt = pmt.tile([128, NT], BF16, tag="mt512")
            nc.scalar.activation(qTmt[:], pq_[:], AF.Copy)
            qsqmt = pmt.tile([128, NT], BF16, tag="mt512")
            nc.scalar.square(qsqmt[:], pq_[:])
            qq[pr] = (qTmt, qsqmt)

        def qsmall(pr):
            qTmt, qsqmt = qq[pr]
            pqs = PR.tile([2, NT], F32, tag="prow")
            nc.tensor.matmul(pqs[:], cAPs["sel2"][:], qsqmt[:], start=True,
                             stop=True)
            qsq2 = psm.tile([2, NT], BF16, tag="qsq2")
            nc.scalar.activation(qsq2[:], pqs[:], AF.Copy)

            pdq = PP.tile([128, NT], F32, tag="pbig")
            nc.tensor.matmul(pdq[:], cAPs["projbd"][:], qTmt[:], start=True,
                             stop=False)
            nc.tensor.matmul(pdq[:], cAPs["negselF"][:], qsq2[:], start=False,
                             stop=True, skip_group_check=True)
            Etmp = pmt.tile([128, NT], BF16, tag="t512bf")
            nc.scalar.activation(Etmp[:], pdq[:], AF.Exp)

            pS = PR.tile([2, NT], F32, tag="prow")
            nc.tensor.matmul(pS[:], cAPs["sel2"][:], Etmp[:], start=True, stop=True)
            ediag = psm.tile([2, NT], BF16, tag="ediag")
            nc.scalar.activation(ediag[:], qsq2[:], AF.Exp, scale=float(DN2H))
            wrow = psm.tile([2, NT], BF16, tag="wrow")
            nc.vector.scalar_tensor_tensor(wrow[:], ediag[:], EPS_KERN, pS[:],
                                           op0=ALU.mult, op1=ALU.mult)
            pwB = PP.tile([128, NT], F32, tag="pbig")
            nc.tensor.matmul(pwB[:], sel2b_bf[:], wrow[:], start=True, stop=True)
            if fold:
                EqU = psm.tile([128, NT], BF16, tag="EqU")
                nc.vector.tensor_tensor(EqU[:], Etmp[:], pwB[:], op=ALU.add)
                kvB = kvB_all[b * PAIRS + pr]
                pden = PR.tile([2, NT], F32, tag="prow")
                nc.tensor.matmul(pden[:], kvB[:, 128:130], EqU[:], start=True,
                                 stop=True)
                rdf = psm.tile([2, NT], F32, tag="rdf")
                nc.vector.reciprocal_approx_fast(rdf[:], pden[:])
                rdb = psm.tile([2, NT], BF16, tag="rdb")
                nc.vector.tensor_copy(rdb[:], rdf[:])
                prdB = PP.tile([128, NT], F32, tag="pbig")
                nc.tensor.matmul(prdB[:], sel2b_bf[:], rdb[:], start=True,
                                 stop=True)
                Eq = peq.tile([128, NT], BF16, tag=f"Eq{pr}")
                nc.vector.tensor_tensor(Eq[:], EqU[:], prdB[:], op=ALU.mult)
            else:
                Eq = peq.tile([128, NT], BF16, tag=f"Eq{pr}")
                nc.vector.tensor_tensor(Eq[:], Etmp[:], pwB[:], op=ALU.add)
            Eqs.append(Eq)

        qproj(0)
        for pr in range(PAIRS):
            if pr + 1 < PAIRS:
                qproj(pr + 1)
            qsmall(pr)
        Eq_all[b] = Eqs

    emit_qside(0)

    # ---- kv / ksum assembly ----
    mx8 = pone.tile([1, 1], F32, tag="mx8")
    nc.vector.tensor_reduce(mx8[:], arresS[0:1, AC_U:AC_U + NC], axis=AX.X,
                           op=ALU.max)
    emxf = pone.tile([1, 1], F32, tag="emxf")
    nc.scalar.activation(emxf[:], mx8[:], AF.Exp, scale=-1.0)
    emxrow = pln.tile([1, NT], F32R, tag="lnrowA")
    nc.vector.tensor_scalar(emxrow[:], onesrow_c[:], emxf[:], None, op0=ALU.mult)
    pex = PP.tile([128, NT], F32, tag="pbig")
    nc.tensor.matmul(pex[:], ones1x128_r[:], emxrow[:], start=True, stop=True)
    emxc = pone.tile([128, 1], F32, tag="emxc")
    nc.vector.tensor_copy(emxc[:], pex[:, 0:1])

    epsv_sb = pbig.tile([128, B * PAIRS, 64], BF16, tag="big32")
    nc.sync.dma_start(epsv_sb[:], epsvA[:])

    kvBall = pone.tile([128, B * PAIRS, 130], BF16, tag="kvBall")
    nc.vector.memset(kvBall[:], 0.0)
    arA = arresA[:].rearrange("p (j c) -> p j c", c=64)
    nc.vector.scalar_tensor_tensor(
        kvBall[0:64, :, 0:64], arA[0:64], emxc[0:64, :], epsv_sb[0:64],
        op0=ALU.mult, op1=ALU.add)
    nc.vector.scalar_tensor_tensor(
        kvBall[64:128, :, 64:128], arA[64:128], emxc[64:128, :], epsv_sb[64:128],
        op0=ALU.mult, op1=ALU.add)
    ksf_all = psm.tile([128, B * PAIRS], F32, tag="ksf")
    nc.vector.tensor_scalar(ksf_all[:], arresS[:, 0:B * PAIRS],
                            emxc[:], float(EPS_KERN * N),
                            op0=ALU.mult, op1=ALU.add)
    nc.vector.tensor_copy(kvBall[0:64, :, 128:129],
                          ksf_all[0:64].unsqueeze(-1))
    nc.vector.tensor_copy(kvBall[64:128, :, 129:130],
                          ksf_all[64:128].unsqueeze(-1))
    kvB_all = {}
    for b in range(B):
        for pr in range(PAIRS):
            j = b * PAIRS + pr
            kvB_all[j] = kvBall[:, j, :]

    # ================= stage B =================
    attnT_all = {}

    emit_qside(1, fold=True)

    def attn_phase_full(b):
        """Batch whose Eq is unscaled: divide by den on the output side."""
        Eqs = Eq_all[b]
        attnT = pbig2.tile([128, KT_D, NT], BF16, tag="bf8")
        for pr in range(PAIRS):
            kvB = kvB_all[b * PAIRS + pr]
            pnum = PP.tile([128, NT], F32, tag="pbig")
            nc.tensor.matmul(pnum[:], kvB[:, 0:128], Eqs[pr][:], start=True,
                             stop=True)
            pden = PR.tile([2, NT], F32, tag="prow")
            nc.tensor.matmul(pden[:], kvB[:, 128:130], Eqs[pr][:], start=True,
                             stop=True)
            rdf = psm.tile([2, NT], F32, tag="rdf")
            nc.vector.reciprocal_approx_fast(rdf[:], pden[:])
            rdb = psm.tile([2, NT], BF16, tag="rdb")
            nc.vector.tensor_copy(rdb[:], rdf[:])
            prdB = PP.tile([128, NT], F32, tag="pbig")
            nc.tensor.matmul(prdB[:], sel2b_bf[:], rdb[:], start=True, stop=True)
            numsb = psm.tile([128, NT], BF16, tag="numsb")
            nc.vector.tensor_copy(numsb[:], pnum[:])
            nc.vector.tensor_tensor(attnT[:, pr, :], numsb[:], prdB[:], op=ALU.mult)
        attnT_all[b] = attnT

    def attn_phase(b):
        """Batch with den pre-folded into Eq: one MM + one copy per pair."""
        Eqs = Eq_all[b]
        attnT = pbig2.tile([128, KT_D, NT], BF16, tag="bf8")
        for pr in range(PAIRS):
            kvB = kvB_all[b * PAIRS + pr]
            pnum = PP.tile([128, NT], F32, tag="pbig")
            nc.tensor.matmul(pnum[:], kvB[:, 0:128], Eqs[pr][:], start=True,
                             stop=True)
            nc.vector.tensor_copy(attnT[:, pr, :], pnum[:])
        attnT_all[b] = attnT

    attn_phase_full(0)
    for b in range(B):
        attnT = attnT_all[b]

        res1 = pbig.tile([128, KT_D, NT], BF16, tag="resX")
        for mt in range(KT_D):
            womt = pstream.tile([128, KT_D, 128], BF16, tag="wmt")
            nc.sync.dma_start(womt[:], wos[mt])
            po = PP.tile([128, NT], F32, tag="pbig")
            for kt in range(KT_D):
                nc.tensor.matmul(po[:], womt[:, kt, :], attnT[:, kt, :],
                                 start=kt == 0, stop=kt == KT_D - 1)
            xf = psm.tile([128, NT], F32, tag="xf")
            nc.sync.dma_start(xf[:], x_f32[b, mt * 128:(mt + 1) * 128, :])
            nc.vector.tensor_tensor(res1[:, mt, :], xf[:], po[:], op=ALU.add)

        out1 = ln(res1, cAPs["g1c"], cAPs["be1c"], "o1", mybir.dt.bfloat16)

        if b + 2 < B:
            emit_qside(b + 2, fold=True)

        hsb = pbig.tile([128, MT_FF, NT], BF16, tag="big32")
        for mt in range(MT_FF):
            w1mt = pstream.tile([128, KT_D, 128], BF16, tag="wmt")
            nc.sync.dma_start(w1mt[:], w1s[mt])
            pz = PP.tile([128, NT], F32, tag="pbig")
            for kt in range(KT_D):
                nc.tensor.matmul(pz[:], w1mt[:, kt, :], out1[:, kt, :],
                                 start=kt == 0, stop=kt == KT_D - 1)
            eraw = pmt.tile([128, NT], BF16, tag="t512bf")
            nc.scalar.activation(eraw[:], pz[:], AF.Exp,
                                 bias=cAPs["b1c"][:, mt:mt + 1])
            emin = pmt.tile([128, NT], BF16, tag="t512bf")
            nc.vector.tensor_scalar(emin[:], eraw[:], 1.0, None, op0=ALU.min)
            nc.vector.scalar_tensor_tensor(hsb[:, mt, :], pz[:],
                                           cAPs["b1p1c"][:, mt:mt + 1], emin[:],
                                           op0=ALU.add, op1=ALU.max)

        res2 = pbig.tile([128, KT_D, NT], BF16, tag="resX")
        for mt in range(KT_D):
            w2mt = pw2s.tile([128, MT_FF, 128], BF16, tag="w2mt")
            nc.sync.dma_start(w2mt[:], w2s[mt])
            pf = PP.tile([128, NT], F32, tag="pbig")
            for kt in range(MT_FF):
                nc.tensor.matmul(pf[:], w2mt[:, kt, :], hsb[:, kt, :],
                                 start=kt == 0, stop=kt == MT_FF - 1)
            nc.vector.scalar_tensor_tensor(res2[:, mt, :], pf[:],
                                           cAPs["b2adjc"][:, mt:mt + 1],
                                           out1[:, mt, :], op0=ALU.add, op1=ALU.add)

        if b + 1 < B:
            attn_phase(b + 1)

        ln(res2, cAPs["g2c"], cAPs["be2c"], "o2", None, dma_out=(out, b))

    ctx.close()


_CACHE = {}


def _build():
    import concourse.tile as tile
    from concourse import bacc
    nc = bacc.Bacc("TRN2", target_bir_lowering=False, debug=False, num_devices=NC)
    with tile.TileContext(nc) as tc:
        _emit(nc, tc)
    nc.compile()
    return nc


def _host_inputs(x, Wq, Wk, Wv, Wo, proj, W1, b1, W2, b2,
                 ln1_g, ln1_b, ln2_g, ln2_b):
    bf = ml_dtypes.bfloat16
    f32 = np.float32
    d = {}

    def chunked(w):  # [D, X] -> [X/128 mt, 128 p, D/128 kt, 128]
        Dk, X = w.shape
        r = w.reshape(Dk // 128, 128, X // 128, 128)
        return np.ascontiguousarray(r.transpose(2, 1, 0, 3)).astype(bf)

    d["wqs"] = chunked(Wq.reshape(D, D))
    d["wks"] = chunked(Wk.reshape(D, D))
    d["wv"] = np.ascontiguousarray(Wv.reshape(D, D)).astype(bf)
    d["wos"] = chunked(Wo.reshape(D, D))
    d["w1s"] = chunked(W1)
    d["w2s"] = chunked(W2)

    projT_s = (proj * DN).T.astype(f32)
    pbd = np.zeros((128, 128), f32)
    pbd[0:64, 0:64] = projT_s
    pbd[64:128, 64:128] = projT_s
    d["projbd"] = pbd.astype(bf)
    nsF = np.zeros((2, 128), f32)
    nsF[0, 0:64] = -DN2H
    nsF[1, 64:128] = -DN2H
    d["negselF"] = nsF.astype(bf)
    s2 = np.zeros((128, 2), f32)
    s2[0:64, 0] = 1.0
    s2[64:128, 1] = 1.0
    d["sel2"] = s2.astype(bf)
    s2b = np.zeros((2, 128), f32)
    s2b[0, 0:64] = 1.0
    s2b[1, 64:128] = 1.0
    d["sel2b"] = s2b
    d["ones128"] = np.ones((128, 1), f32).astype(bf)
    d["ones1x128"] = np.ones((1, 128), f32)
    d["mean1"] = np.full((128, 1), 1.0 / D, f32)
    hm2 = np.zeros((128, 2), f32)
    hm2[0:64, 0] = 1.0
    hm2[64:128, 1] = 1.0
    d["headmask2"] = hm2

    xsum = x.sum(axis=1, dtype=np.float64)
    vsum = xsum @ Wv.reshape(D, D).astype(np.float64)
    epsv = (EPS_KERN * vsum).astype(f32)            # [B, D]
    # packed [128, B*PAIRS, 64]: rows 0:64 head0 slice, rows 64:128 head1
    epsvA = np.zeros((128, B * PAIRS, 64), f32)
    ev = epsv.reshape(B, PAIRS, 2, 64)
    for b in range(B):
        for pr in range(PAIRS):
            epsvA[0:64, b * PAIRS + pr, :] = ev[b, pr, 0][None, :]
            epsvA[64:128, b * PAIRS + pr, :] = ev[b, pr, 1][None, :]
    d["epsvA"] = epsvA.astype(bf)

    d["b1c"] = np.ascontiguousarray(b1.reshape(MT_FF, 128).T).astype(f32)
    d["b1p1c"] = np.ascontiguousarray((b1 + 1.0).reshape(MT_FF, 128).T).astype(f32)
    b2adj = b2.astype(np.float64) - W2.astype(np.float64).sum(axis=0)
    d["b2adjc"] = np.ascontiguousarray(b2adj.reshape(KT_D, 128).T).astype(f32)
    d["g1c"] = np.ascontiguousarray(ln1_g.reshape(KT_D, 128).T).astype(f32)
    d["be1c"] = np.ascontiguousarray(ln1_b.reshape(KT_D, 128).T).astype(f32)
    d["g2c"] = np.ascontiguousarray(ln2_g.reshape(KT_D, 128).T).astype(f32)
    d["be2c"] = np.ascontiguousarray(ln2_b.reshape(KT_D, 128).T).astype(f32)
    return d


def kernel(x, Wq, Wk, Wv, Wo, proj, W1, b1, W2, b2, ln1_g, ln1_b, ln2_g, ln2_b):
    from concourse import bass_utils

    x = np.asarray(x, np.float32)
    shared = _host_inputs(x, np.asarray(Wq), np.asarray(Wk), np.asarray(Wv),
                          np.asarray(Wo), np.asarray(proj), np.asarray(W1),
                          np.asarray(b1), np.asarray(W2), np.asarray(b2),
                          np.asarray(ln1_g), np.asarray(ln1_b),
                          np.asarray(ln2_g), np.asarray(ln2_b))

    if "nc" not in _CACHE:
        _CACHE["nc"] = _build()
    nc = _CACHE["nc"]

    in_maps = []
    for c in range(NC):
        xs = x[:, c * NT:(c + 1) * NT, :]
        xT = np.ascontiguousarray(xs.transpose(0, 2, 1))
        oh = np.zeros((1, NC), np.float32)
        oh[0, c] = 1.0
        m = dict(shared)
        m["x_f32"] = xT
        m["x_bf"] = xT.astype(ml_dtypes.bfloat16)
        m["onehot"] = oh
        in_maps.append(m)

    trace = bool(int(os.environ.get("KERNEL_TRACE", "0")))
    res = bass_utils.run_bass_kernel_spmd(nc, in_maps, core_ids=list(range(NC)),
                                          trace=trace)
    if trace and res.exec_time_ns is not None:
        print(f"HW exec time: {res.exec_time_ns} ns")
        if res.instructions_and_trace is not None:
            print("trace:", res.instructions_and_trace[1])

    outp = np.empty((B, N, D), np.float32)
    for c in range(NC):
        oT = res.results[c]["out"]
        outp[:, c * NT:(c + 1) * NT, :] = oT.transpose(0, 2, 1)
    return outp



# revision 34
# speedup vs baseline: 1.0383x; 1.0383x over previous
"""Performer (FAVOR+) encoder layer on 8 trn2 NeuronCores.

Sharding: data-parallel over sequence (512 positions per core x 4 batches).
The linear-attention summaries (A = E_k^T v per (batch, head), usum) and the
global key-feature max (via one-hot slots) are combined in ONE packed
AllReduce, overlapped with the Q-side feature compute.

Layout: activations feature-major (xT = [D, tokens]) so every GEMM's
stationary operand is a natural weight slice; per-token reductions and
broadcasts are small PE matmuls (selector / ones / f32r broadcast matmuls).
E_k and v are produced token-major directly by matmuls so the token-
contraction A-matmul needs no transposes.
"""
import os
import numpy as np
import ml_dtypes

B, N, D = 4, 4096, 1024
H, DH = 16, 64
DFF = 4096
M = 64
EPS_KERN = 1e-6
EPS_LN = 1e-6
NC = 8
NT = N // NC                # 512 positions per core per batch
PAIRS = H // 2              # 8 head-pairs
KT_D = D // 128             # 8
MT_FF = DFF // 128          # 32
TT = NT // 128              # 4
DN = 1.0 / np.sqrt(np.sqrt(DH))
DN2H = DN * DN / 2.0


def _emit(nc, tc):
    import concourse.mybir as mybir
    from contextlib import ExitStack
    F32 = mybir.dt.float32
    F32R = mybir.dt.float32r
    BF16 = mybir.dt.bfloat16
    AF = mybir.ActivationFunctionType
    ALU = mybir.AluOpType
    AX = mybir.AxisListType

    dram = lambda name, shape, dt, kind: nc.dram_tensor(name, shape, dt, kind=kind).ap()

    x_bf = dram("x_bf", [B, D, NT], BF16, "ExternalInput")
    x_f32 = dram("x_f32", [B, D, NT], F32, "ExternalInput")
    wqs = dram("wqs", [KT_D, 128, KT_D, 128], BF16, "ExternalInput")
    wks = dram("wks", [KT_D, 128, KT_D, 128], BF16, "ExternalInput")
    wv = dram("wv", [D, D], BF16, "ExternalInput")
    wos = dram("wos", [KT_D, 128, KT_D, 128], BF16, "ExternalInput")
    w1s = dram("w1s", [MT_FF, 128, KT_D, 128], BF16, "ExternalInput")
    w2s = dram("w2s", [KT_D, 128, MT_FF, 128], BF16, "ExternalInput")
    projbd = dram("projbd", [128, 128], BF16, "ExternalInput")
    negselF = dram("negselF", [2, 128], BF16, "ExternalInput")
    sel2 = dram("sel2", [128, 2], BF16, "ExternalInput")
    sel2b = dram("sel2b", [2, 128], F32, "ExternalInput")
    ones128 = dram("ones128", [128, 1], BF16, "ExternalInput")
    ones1x128 = dram("ones1x128", [1, 128], F32, "ExternalInput")
    mean1 = dram("mean1", [128, 1], F32, "ExternalInput")
    headmask2 = dram("headmask2", [128, 2], F32, "ExternalInput")
    epsvA = dram("epsvA", [128, B * PAIRS, 64], BF16, "ExternalInput")
    onehot = dram("onehot", [1, NC], F32, "ExternalInput")
    b1c = dram("b1c", [128, MT_FF], F32, "ExternalInput")
    b1p1c = dram("b1p1c", [128, MT_FF], F32, "ExternalInput")
    b2adjc = dram("b2adjc", [128, KT_D], F32, "ExternalInput")
    g1c = dram("g1c", [128, KT_D], F32, "ExternalInput")
    be1c = dram("be1c", [128, KT_D], F32, "ExternalInput")
    g2c = dram("g2c", [128, KT_D], F32, "ExternalInput")
    be2c = dram("be2c", [128, KT_D], F32, "ExternalInput")
    out = dram("out", [B, D, NT], F32, "ExternalOutput")

    AC_A = B * PAIRS * 64
    AC_U = B * PAIRS
    AC = AC_A + AC_U + NC

    ctx = ExitStack()
    pconst = ctx.enter_context(tc.tile_pool(name="pconst", bufs=1))
    pwv = ctx.enter_context(tc.tile_pool(name="pwv", bufs=1))
    pstream = ctx.enter_context(tc.tile_pool(name="pstream", bufs=2))
    pw2s = ctx.enter_context(tc.tile_pool(name="pw2s", bufs=2))
    pxa = ctx.enter_context(tc.tile_pool(name="pxa", bufs=1))
    pxb = ctx.enter_context(tc.tile_pool(name="pxb", bufs=1))
    pmt = ctx.enter_context(tc.tile_pool(name="pmt", bufs=3))
    psm = ctx.enter_context(tc.tile_pool(name="psm", bufs=2))
    pln = ctx.enter_context(tc.tile_pool(name="pln", bufs=1))
    peq = ctx.enter_context(tc.tile_pool(name="peq", bufs=2))
    pbig = ctx.enter_context(tc.tile_pool(name="pbig", bufs=1))
    pbig2 = ctx.enter_context(tc.tile_pool(name="pbig2", bufs=2))
    pres = ctx.enter_context(tc.tile_pool(name="pres", bufs=2))
    pone = ctx.enter_context(tc.tile_pool(name="pone", bufs=1))
    pdram = ctx.enter_context(tc.tile_pool(name="pdram", bufs=1, space="DRAM"))
    PP = ctx.enter_context(tc.tile_pool(name="PP", bufs=3, space="PSUM"))
    PD = ctx.enter_context(tc.tile_pool(name="PD", bufs=2, space="PSUM"))
    PA_ = ctx.enter_context(tc.tile_pool(name="PA", bufs=1, space="PSUM"))
    PR = ctx.enter_context(tc.tile_pool(name="PR", bufs=1, space="PSUM"))
    PB = ctx.enter_context(tc.tile_pool(name="PB", bufs=1, space="PSUM"))

    # ---- constants ----
    wv_sb = pwv.tile([128, KT_D, D], BF16, tag="wv")
    nc.sync.dma_start(wv_sb[:], wv.rearrange("(kt p) m -> p kt m", p=128))
    cAPs = {}
    for name, ap, shape, dt in (
        ("projbd", projbd, [128, 128], BF16), ("negselF", negselF, [2, 128], BF16),
        ("sel2", sel2, [128, 2], BF16), ("sel2b", sel2b, [2, 128], F32),
        ("ones128", ones128, [128, 1], BF16), ("ones1x128", ones1x128, [1, 128], F32),
        ("mean1", mean1, [128, 1], F32), ("headmask2", headmask2, [128, 2], F32),
        ("onehot", onehot, [1, NC], F32), ("b1c", b1c, [128, MT_FF], F32),
        ("b1p1c", b1p1c, [128, MT_FF], F32), ("b2adjc", b2adjc, [128, KT_D], F32),
        ("g1c", g1c, [128, KT_D], F32), ("be1c", be1c, [128, KT_D], F32),
        ("g2c", g2c, [128, KT_D], F32), ("be2c", be2c, [128, KT_D], F32),
    ):
        t = pconst.tile(shape, dt, tag=name)
        nc.sync.dma_start(t[:], ap[:])
        cAPs[name] = t
    sel2b_r = pconst.tile([2, 128], F32R, tag="sel2br")
    ones1x128_r = pconst.tile([1, 128], F32R, tag="ones1x128r")
    mean1_r = pconst.tile([128, 1], F32R, tag="mean1r")
    mean1_bf = pconst.tile([128, 1], BF16, tag="mean1bf")
    sel2b_bf = pconst.tile([2, 128], BF16, tag="sel2bbf")
    nc.vector.tensor_copy(sel2b_r[:], cAPs["sel2b"][:])
    nc.vector.tensor_copy(ones1x128_r[:], cAPs["ones1x128"][:])
    nc.vector.tensor_copy(mean1_r[:], cAPs["mean1"][:])
    nc.vector.tensor_copy(mean1_bf[:], cAPs["mean1"][:])
    nc.vector.tensor_copy(sel2b_bf[:], cAPs["sel2b"][:])

    epsln_c = pconst.tile([1, 1], F32, tag="epslnc")
    nc.vector.memset(epsln_c[:], float(EPS_LN))
    onesrow_c = pconst.tile([1, NT], F32, tag="onesrowc")
    nc.vector.memset(onesrow_c[:], 1.0)
    ones1x128_bf = pconst.tile([1, 128], BF16, tag="ones1x128bf")
    nc.vector.tensor_copy(ones1x128_bf[:], cAPs["ones1x128"][:])
    arstA = pone.tile([128, AC_A], BF16, tag="arbufA")
    arstS = pone.tile([128, AC_U + NC], F32, tag="arbufS")
    mxcols = pone.tile([128, B * PAIRS * TT], BF16, tag="mxcols")

    def ln(res, gc, bc, tag, odt, dma_out=None):
        """Feature-major LN over a bf16 [128, KT_D, NT] residual tile.
        Stats via bf16 PE reductions; mu/rstd broadcast to all partitions
        via K=1 matmuls, evacuated to bf16 so the per-kt chain runs at
        DVE 2x rate."""
        pm0 = PR.tile([1, NT], F32, tag="prow")
        pm1 = PA_.tile([1, NT], F32, tag="pA")
        for kt in range(KT_D):
            sqt = pln.tile([128, NT], BF16, tag="lnsq")
            nc.scalar.square(sqt[:], res[:, kt, :])
            nc.tensor.matmul(pm0[:], mean1_bf[:], res[:, kt, :],
                             start=kt == 0, stop=kt == KT_D - 1,
                             skip_group_check=True)
            nc.tensor.matmul(pm1[:], mean1_bf[:], sqt[:],
                             start=kt == 0, stop=kt == KT_D - 1,
                             skip_group_check=True)
        mu = pln.tile([1, NT], BF16, tag="lnmu")
        nc.vector.tensor_copy(mu[:], pm0[:])
        mu2 = pln.tile([1, NT], F32, tag="lnrowA")
        nc.vector.tensor_tensor(mu2[:], mu[:], mu[:], op=ALU.mult)
        var = pln.tile([1, NT], F32, tag="lnrowB")
        nc.vector.tensor_tensor(var[:], pm1[:], mu2[:], op=ALU.subtract)
        lnv = pln.tile([1, NT], F32, tag="lnrowA")
        nc.scalar.activation(lnv[:], var[:], AF.Ln, bias=epsln_c[:])
        rstdf = pln.tile([1, NT], BF16, tag="lnrowB2")
        nc.scalar.activation(rstdf[:], lnv[:], AF.Exp, scale=-0.5)
        pmu = PB.tile([128, NT], F32, tag="pbc")
        nc.tensor.matmul(pmu[:], ones1x128_bf[:], mu[:], start=True, stop=True)
        muB = pln.tile([128, NT], BF16, tag="lnmuB")
        nc.vector.tensor_copy(muB[:], pmu[:])
        prs = PB.tile([128, NT], F32, tag="pbc")
        nc.tensor.matmul(prs[:], ones1x128_bf[:], rstdf[:], start=True, stop=True)
        rsB = pln.tile([128, NT], BF16, tag="lnrsB")
        nc.vector.tensor_copy(rsB[:], prs[:])
        o = None
        if odt is not None:
            o = pbig2.tile([128, KT_D, NT], odt, tag="bf8")
        cen = pln.tile([128, NT], BF16, tag="lncen")
        nrm = pln.tile([128, NT], BF16, tag="lnnrm")
        for kt in range(KT_D):
            nc.vector.tensor_tensor(cen[:], res[:, kt, :], muB[:],
                                    op=ALU.subtract)
            nc.vector.tensor_tensor(nrm[:], cen[:], rsB[:], op=ALU.mult)
            if o is not None:
                nc.vector.tensor_scalar(o[:, kt, :], nrm[:], gc[:, kt:kt + 1],
                                        bc[:, kt:kt + 1], op0=ALU.mult, op1=ALU.add)
            else:
                ot = pln.tile([128, NT], F32, tag="lnot")
                nc.vector.tensor_scalar(ot[:], nrm[:], gc[:, kt:kt + 1],
                                        bc[:, kt:kt + 1], op0=ALU.mult, op1=ALU.add)
                nc.sync.dma_start(dma_out[0][dma_out[1], kt * 128:(kt + 1) * 128, :],
                                  ot[:])
        return o

    # ================= stage A =================
    vtok = pxa.tile([128, TT, PAIRS, 129], BF16, tag="vtok")
    nc.vector.memset(vtok[:], 1.0)
    NB_A = PAIRS * 64           # AR payload columns per batch

    def fire_ar(tag, src_ap, ncols, dt=None):
        dt = dt or F32
        ain = pdram.tile([128, ncols], dt, tag=f"arin{tag}")
        aout = pdram.tile([128, ncols], dt, tag=f"arout{tag}",
                          addr_space="Shared")
        nc.sync.dma_start(ain[:], src_ap)
        if os.environ.get("KERNEL_NOCOLL"):
            nc.sync.dma_start(aout[:], ain[:])
        else:
            nc.gpsimd.collective_compute("AllReduce", ALU.add,
                                         replica_groups=[list(range(NC))],
                                         ins=[ain[:]], outs=[aout[:]])
        return aout

    ar_outs = {}
    for b in range(B):
        xbf = pxb.tile([128, KT_D, NT], BF16, tag="xbf")
        nc.sync.dma_start(xbf[:], x_bf[b].rearrange("(kt p) t -> p kt t", p=128))

        for tt in range(TT):
            for nh in range(2):
                pv = PP.tile([128, NT], F32, tag="pbig")
                for kt in range(KT_D):
                    nc.tensor.matmul(pv[:], xbf[:, kt, tt * 128:(tt + 1) * 128],
                                     wv_sb[:, kt, nh * 512:(nh + 1) * 512],
                                     start=kt == 0, stop=kt == KT_D - 1)
                for i in range(4):
                    nc.scalar.activation(vtok[:, tt, nh * 4 + i, 0:128],
                                         pv[:, i * 128:(i + 1) * 128], AF.Copy)

        kq = {}

        def kproj(pr):
            wkmt = pstream.tile([128, KT_D, 128], BF16, tag="wmt")
            nc.sync.dma_start(wkmt[:], wks[pr])
            pk = PP.tile([128, NT], F32, tag="pbig")
            for kt in range(KT_D):
                nc.tensor.matmul(pk[:], wkmt[:, kt, :], xbf[:, kt, :],
                                 start=kt == 0, stop=kt == KT_D - 1)
            kTmt = pmt.tile([128, NT], BF16, tag="mt512")
            nc.scalar.activation(kTmt[:], pk[:], AF.Copy)
            ksqmt = pmt.tile([128, NT], BF16, tag="mt512")
            nc.scalar.square(ksqmt[:], pk[:])
            kq[pr] = (kTmt, ksqmt)

        def ksmall(pr):
            kTmt, ksqmt = kq[pr]
            pks = PR.tile([2, NT], F32, tag="prow")
            nc.tensor.matmul(pks[:], cAPs["sel2"][:], ksqmt[:], start=True,
                             stop=True)
            ksq2 = psm.tile([2, NT], BF16, tag="ksq2")
            nc.scalar.activation(ksq2[:], pks[:], AF.Copy)

            Ek = psm.tile([128, TT, 128], BF16, tag="Ek")
            for tt in range(TT):
                pdd = PD.tile([128, 128], F32, tag="pdd")
                nc.tensor.matmul(pdd[:], kTmt[:, tt * 128:(tt + 1) * 128],
                                 cAPs["projbd"][:], start=True, stop=True)
                c = (b * PAIRS + pr) * TT + tt
                nc.vector.tensor_reduce(mxcols[:, c:c + 1], pdd[:], axis=AX.X,
                                        op=ALU.max)
                nc.tensor.matmul(pdd[:], ksq2[:, tt * 128:(tt + 1) * 128],
                                 cAPs["negselF"][:], start=False, stop=True,
                                 skip_group_check=True)
                nc.scalar.activation(Ek[:, tt, :], pdd[:], AF.Exp)

            pA = PA_.tile([128, 129], F32, tag="pA")
            for tt in range(TT):
                nc.tensor.matmul(pA[:], Ek[:, tt, :], vtok[:, tt, pr, :],
                                 start=tt == 0, stop=tt == TT - 1,
                                 skip_group_check=True)
            j = b * PAIRS + pr
            nc.vector.tensor_copy(arstA[0:64, j * 64:(j + 1) * 64],
                                  pA[0:64, 0:64])
            nc.vector.tensor_copy(arstA[64:128, j * 64:(j + 1) * 64],
                                  pA[64:128, 64:128])
            nc.vector.tensor_copy(arstS[:, j:j + 1], pA[:, 128:129])

        kproj(0)
        for pr in range(PAIRS):
            if pr + 1 < PAIRS:
                kproj(pr + 1)
            ksmall(pr)

        ar_outs[b] = fire_ar(f"A{b}", arstA[:, b * NB_A:(b + 1) * NB_A],
                             NB_A, BF16)

    # ---- fire final (usum + max) AllReduce ----
    mxr = pone.tile([128, 1], F32, tag="mxr")
    nc.vector.tensor_reduce(mxr[:], mxcols[:], axis=AX.X, op=ALU.max)
    mx1 = pone.tile([1, 1], F32, tag="mx1")
    nc.gpsimd.tensor_reduce(mx1[:], mxr[:], axis=AX.C, op=ALU.max)
    nc.vector.tensor_scalar(arstS[0:1, AC_U:AC_U + NC], cAPs["onehot"][:],
                            mx1[:], None, op0=ALU.mult)
    ar_small = fire_ar("S", arstS[:], AC_U + NC)

    arresA = pone.tile([128, AC_A], BF16, tag="arbufA")
    arresS = pone.tile([128, AC_U + NC], F32, tag="arbufS")
    for b in range(B):
        nc.sync.dma_start(arresA[:, b * NB_A:(b + 1) * NB_A], ar_outs[b][:])
    nc.sync.dma_start(arresS[:], ar_small[:])

    # ================= q-side features (overlap AR) =================
    Eq_all = {}

    def emit_qside(b, fold=False):
        qxbf = pxa.tile([128, KT_D, NT], BF16, tag="qxbf")
        nc.sync.dma_start(qxbf[:], x_bf[b].rearrange("(kt p) t -> p kt t", p=128))
        Eqs = []
        qq = {}

        def qproj(pr):
            wqmt = pstream.tile([128, KT_D, 128], BF16, tag="wmt")
            nc.sync.dma_start(wqmt[:], wqs[pr])
            pq_ = PP.tile([128, NT], F32, tag="pbig")
            for kt in range(KT_D):
                nc.tensor.matmul(pq_[:], wqmt[:, kt, :], qxbf[:, kt, :],
                                 start=kt == 0, stop=kt == KT_D - 1)
            qTmt = pmt.tile([128, NT], BF16, tag="mt512")
            nc.scalar.activation(qTmt[:], pq_[:], AF.Copy)
            qsqmt = pmt.tile([128, NT], BF16, tag="mt512")
            nc.scalar.square(qsqmt[:], pq_[:])
            qq[pr] = (qTmt, qsqmt)

        def qsmall(pr):
            qTmt, qsqmt = qq[pr]
            pqs = PR.tile([2, NT], F32, tag="prow")
            nc.tensor.matmul(pqs[:], cAPs["sel2"][:], qsqmt[:], start=True,
                             stop=True)
            qsq2 = psm.tile([2, NT], BF16, tag="qsq2")
            nc.scalar.activation(qsq2[:], pqs[:], AF.Copy)

            pdq = PP.tile([128, NT], F32, tag="pbig")
            nc.tensor.matmul(pdq[:], cAPs["projbd"][:], qTmt[:], start=True,
                             stop=False)
            nc.tensor.matmul(pdq[:], cAPs["negselF"][:], qsq2[:], start=False,
                             stop=True, skip_group_check=True)
            Etmp = pmt.tile([128, NT], BF16, tag="t512bf")
            nc.scalar.activation(Etmp[:], pdq[:], AF.Exp)

            pS = PR.tile([2, NT], F32, tag="prow")
            nc.tensor.matmul(pS[:], cAPs["sel2"][:], Etmp[:], start=True, stop=True)
            ediag = psm.tile([2, NT], BF16, tag="ediag")
            nc.scalar.activation(ediag[:], qsq2[:], AF.Exp, scale=float(DN2H))
            wrow = psm.tile([2, NT], BF16, tag="wrow")
            nc.vector.scalar_tensor_tensor(wrow[:], ediag[:], EPS_KERN, pS[:],
                                           op0=ALU.mult, op1=ALU.mult)
            pwB = PB.tile([128, NT], F32, tag="pbc")
            nc.tensor.matmul(pwB[:], sel2b_bf[:], wrow[:], start=True, stop=True)
            if fold:
                EqU = psm.tile([128, NT], BF16, tag="EqU")
                nc.vector.tensor_tensor(EqU[:], Etmp[:], pwB[:], op=ALU.add)
                kvB = kvB_all[b * PAIRS + pr]
                pden = PR.tile([2, NT], F32, tag="prow")
                nc.tensor.matmul(pden[:], kvB[:, 128:130], EqU[:], start=True,
                                 stop=True)
                rdf = psm.tile([2, NT], F32, tag="rdf")
                nc.vector.reciprocal_approx_fast(rdf[:], pden[:])
                rdb = psm.tile([2, NT], BF16, tag="rdb")
                nc.vector.tensor_copy(rdb[:], rdf[:])
                prdB = PB.tile([128, NT], F32, tag="pbc")
                nc.tensor.matmul(prdB[:], sel2b_bf[:], rdb[:], start=True,
                                 stop=True)
                Eq = peq.tile([128, NT], BF16, tag=f"Eq{pr}")
                nc.vector.tensor_tensor(Eq[:], EqU[:], prdB[:], op=ALU.mult)
            else:
                Eq = peq.tile([128, NT], BF16, tag=f"Eq{pr}")
                nc.vector.tensor_tensor(Eq[:], Etmp[:], pwB[:], op=ALU.add)
            Eqs.append(Eq)

        qproj(0)
        for pr in range(PAIRS):
            if pr + 1 < PAIRS:
                qproj(pr + 1)
            qsmall(pr)
        Eq_all[b] = Eqs

    emit_qside(0)

    # ---- kv / ksum assembly ----
    mx8 = pone.tile([1, 1], F32, tag="mx8")
    nc.vector.tensor_reduce(mx8[:], arresS[0:1, AC_U:AC_U + NC], axis=AX.X,
                           op=ALU.max)
    emxf = pone.tile([1, 1], F32, tag="emxf")
    nc.scalar.activation(emxf[:], mx8[:], AF.Exp, scale=-1.0)
    emxrow = pln.tile([1, NT], F32R, tag="lnrowA")
    nc.vector.tensor_scalar(emxrow[:], onesrow_c[:], emxf[:], None, op0=ALU.mult)
    pex = PP.tile([128, NT], F32, tag="pbig")
    nc.tensor.matmul(pex[:], ones1x128_r[:], emxrow[:], start=True, stop=True)
    emxc = pone.tile([128, 1], F32, tag="emxc")
    nc.vector.tensor_copy(emxc[:], pex[:, 0:1])

    epsv_sb = pbig.tile([128, B * PAIRS, 64], BF16, tag="big32")
    nc.sync.dma_start(epsv_sb[:], epsvA[:])

    kvBall = pone.tile([128, B * PAIRS, 130], BF16, tag="kvBall")
    nc.vector.memset(kvBall[:], 0.0)
    arA = arresA[:].rearrange("p (j c) -> p j c", c=64)
    nc.vector.scalar_tensor_tensor(
        kvBall[0:64, :, 0:64], arA[0:64], emxc[0:64, :], epsv_sb[0:64],
        op0=ALU.mult, op1=ALU.add)
    nc.vector.scalar_tensor_tensor(
        kvBall[64:128, :, 64:128], arA[64:128], emxc[64:128, :], epsv_sb[64:128],
        op0=ALU.mult, op1=ALU.add)
    ksf_all = psm.tile([128, B * PAIRS], F32, tag="ksf")
    nc.vector.tensor_scalar(ksf_all[:], arresS[:, 0:B * PAIRS],
                            emxc[:], float(EPS_KERN * N),
                            op0=ALU.mult, op1=ALU.add)
    nc.vector.tensor_copy(kvBall[0:64, :, 128:129],
                          ksf_all[0:64].unsqueeze(-1))
    nc.vector.tensor_copy(kvBall[64:128, :, 129:130],
                          ksf_all[64:128].unsqueeze(-1))
    kvB_all = {}
    for b in range(B):
        for pr in range(PAIRS):
            j = b * PAIRS + pr
            kvB_all[j] = kvBall[:, j, :]

    # ================= stage B =================
    attnT_all = {}

    emit_qside(1, fold=True)

    def attn_phase_full(b):
        """Batch whose Eq is unscaled: divide by den on the output side."""
        Eqs = Eq_all[b]
        attnT = pbig2.tile([128, KT_D, NT], BF16, tag="bf8")
        for pr in range(PAIRS):
            kvB = kvB_all[b * PAIRS + pr]
            pnum = PP.tile([128, NT], F32, tag="pbig")
            nc.tensor.matmul(pnum[:], kvB[:, 0:128], Eqs[pr][:], start=True,
                             stop=True)
            pden = PR.tile([2, NT], F32, tag="prow")
            nc.tensor.matmul(pden[:], kvB[:, 128:130], Eqs[pr][:], start=True,
                             stop=True)
            rdf = psm.tile([2, NT], F32, tag="rdf")
            nc.vector.reciprocal_approx_fast(rdf[:], pden[:])
            rdb = psm.tile([2, NT], BF16, tag="rdb")
            nc.vector.tensor_copy(rdb[:], rdf[:])
            prdB = PB.tile([128, NT], F32, tag="pbc")
            nc.tensor.matmul(prdB[:], sel2b_bf[:], rdb[:], start=True, stop=True)
            numsb = psm.tile([128, NT], BF16, tag="numsb")
            nc.vector.tensor_copy(numsb[:], pnum[:])
            nc.vector.tensor_tensor(attnT[:, pr, :], numsb[:], prdB[:], op=ALU.mult)
        attnT_all[b] = attnT

    def attn_phase(b):
        """Batch with den pre-folded into Eq: one MM + one copy per pair."""
        Eqs = Eq_all[b]
        attnT = pbig2.tile([128, KT_D, NT], BF16, tag="bf8")
        for pr in range(PAIRS):
            kvB = kvB_all[b * PAIRS + pr]
            pnum = PP.tile([128, NT], F32, tag="pbig")
            nc.tensor.matmul(pnum[:], kvB[:, 0:128], Eqs[pr][:], start=True,
                             stop=True)
            nc.vector.tensor_copy(attnT[:, pr, :], pnum[:])
        attnT_all[b] = attnT

    res1_all = {}

    def wo_res1(b):
        attnT = attnT_all[b]
        res1 = pres.tile([128, KT_D, NT], BF16, tag="resX")
        for mt in range(KT_D):
            womt = pstream.tile([128, KT_D, 128], BF16, tag="wmt")
            nc.sync.dma_start(womt[:], wos[mt])
            po = PP.tile([128, NT], F32, tag="pbig")
            for kt in range(KT_D):
                nc.tensor.matmul(po[:], womt[:, kt, :], attnT[:, kt, :],
                                 start=kt == 0, stop=kt == KT_D - 1)
            xf = psm.tile([128, NT], F32, tag="xf")
            nc.sync.dma_start(xf[:], x_f32[b, mt * 128:(mt + 1) * 128, :])
            nc.vector.tensor_tensor(res1[:, mt, :], xf[:], po[:], op=ALU.add)
        res1_all[b] = res1

    attn_phase_full(0)
    wo_res1(0)
    for b in range(B):
        out1 = ln(res1_all[b], cAPs["g1c"], cAPs["be1c"], "o1", mybir.dt.bfloat16)

        if b + 2 < B:
            emit_qside(b + 2, fold=True)

        hsb = pbig.tile([128, MT_FF, NT], BF16, tag="big32")
        for mt in range(MT_FF):
            w1mt = pstream.tile([128, KT_D, 128], BF16, tag="wmt")
            nc.sync.dma_start(w1mt[:], w1s[mt])
            pz = PP.tile([128, NT], F32, tag="pbig")
            for kt in range(KT_D):
                nc.tensor.matmul(pz[:], w1mt[:, kt, :], out1[:, kt, :],
                                 start=kt == 0, stop=kt == KT_D - 1)
            eraw = pmt.tile([128, NT], BF16, tag="t512bf")
            nc.scalar.activation(eraw[:], pz[:], AF.Exp,
                                 bias=cAPs["b1c"][:, mt:mt + 1])
            emin = pmt.tile([128, NT], BF16, tag="t512bf")
            nc.vector.tensor_scalar(emin[:], eraw[:], 1.0, None, op0=ALU.min)
            nc.vector.scalar_tensor_tensor(hsb[:, mt, :], pz[:],
                                           cAPs["b1p1c"][:, mt:mt + 1], emin[:],
                                           op0=ALU.add, op1=ALU.max)

        res2 = pres.tile([128, KT_D, NT], BF16, tag="resX")
        for mt in range(KT_D):
            w2mt = pw2s.tile([128, MT_FF, 128], BF16, tag="w2mt")
            nc.sync.dma_start(w2mt[:], w2s[mt])
            pf = PP.tile([128, NT], F32, tag="pbig")
            for kt in range(MT_FF):
                nc.tensor.matmul(pf[:], w2mt[:, kt, :], hsb[:, kt, :],
                                 start=kt == 0, stop=kt == MT_FF - 1)
            nc.vector.scalar_tensor_tensor(res2[:, mt, :], pf[:],
                                           cAPs["b2adjc"][:, mt:mt + 1],
                                           out1[:, mt, :], op0=ALU.add, op1=ALU.add)

        if b + 1 < B:
            attn_phase(b + 1)
            wo_res1(b + 1)

        ln(res2, cAPs["g2c"], cAPs["be2c"], "o2", None, dma_out=(out, b))

    ctx.close()


_CACHE = {}


def _build():
    import concourse.tile as tile
    from concourse import bacc
    nc = bacc.Bacc("TRN2", target_bir_lowering=False, debug=False, num_devices=NC)
    with tile.TileContext(nc) as tc:
        _emit(nc, tc)
    nc.compile()
    return nc


def _host_inputs(x, Wq, Wk, Wv, Wo, proj, W1, b1, W2, b2,
                 ln1_g, ln1_b, ln2_g, ln2_b):
    bf = ml_dtypes.bfloat16
    f32 = np.float32
    d = {}

    def chunked(w):  # [D, X] -> [X/128 mt, 128 p, D/128 kt, 128]
        Dk, X = w.shape
        r = w.reshape(Dk // 128, 128, X // 128, 128)
        return np.ascontiguousarray(r.transpose(2, 1, 0, 3)).astype(bf)

    d["wqs"] = chunked(Wq.reshape(D, D))
    d["wks"] = chunked(Wk.reshape(D, D))
    d["wv"] = np.ascontiguousarray(Wv.reshape(D, D)).astype(bf)
    d["wos"] = chunked(Wo.reshape(D, D))
    d["w1s"] = chunked(W1)
    d["w2s"] = chunked(W2)

    projT_s = (proj * DN).T.astype(f32)
    pbd = np.zeros((128, 128), f32)
    pbd[0:64, 0:64] = projT_s
    pbd[64:128, 64:128] = projT_s
    d["projbd"] = pbd.astype(bf)
    nsF = np.zeros((2, 128), f32)
    nsF[0, 0:64] = -DN2H
    nsF[1, 64:128] = -DN2H
    d["negselF"] = nsF.astype(bf)
    s2 = np.zeros((128, 2), f32)
    s2[0:64, 0] = 1.0
    s2[64:128, 1] = 1.0
    d["sel2"] = s2.astype(bf)
    s2b = np.zeros((2, 128), f32)
    s2b[0, 0:64] = 1.0
    s2b[1, 64:128] = 1.0
    d["sel2b"] = s2b
    d["ones128"] = np.ones((128, 1), f32).astype(bf)
    d["ones1x128"] = np.ones((1, 128), f32)
    d["mean1"] = np.full((128, 1), 1.0 / D, f32)
    hm2 = np.zeros((128, 2), f32)
    hm2[0:64, 0] = 1.0
    hm2[64:128, 1] = 1.0
    d["headmask2"] = hm2

    xsum = x.sum(axis=1, dtype=np.float64)
    vsum = xsum @ Wv.reshape(D, D).astype(np.float64)
    epsv = (EPS_KERN * vsum).astype(f32)            # [B, D]
    # packed [128, B*PAIRS, 64]: rows 0:64 head0 slice, rows 64:128 head1
    epsvA = np.zeros((128, B * PAIRS, 64), f32)
    ev = epsv.reshape(B, PAIRS, 2, 64)
    for b in range(B):
        for pr in range(PAIRS):
            epsvA[0:64, b * PAIRS + pr, :] = ev[b, pr, 0][None, :]
            epsvA[64:128, b * PAIRS + pr, :] = ev[b, pr, 1][None, :]
    d["epsvA"] = epsvA.astype(bf)

    d["b1c"] = np.ascontiguousarray(b1.reshape(MT_FF, 128).T).astype(f32)
    d["b1p1c"] = np.ascontiguousarray((b1 + 1.0).reshape(MT_FF, 128).T).astype(f32)
    b2adj = b2.astype(np.float64) - W2.astype(np.float64).sum(axis=0)
    d["b2adjc"] = np.ascontiguousarray(b2adj.reshape(KT_D, 128).T).astype(f32)
    d["g1c"] = np.ascontiguousarray(ln1_g.reshape(KT_D, 128).T).astype(f32)
    d["be1c"] = np.ascontiguousarray(ln1_b.reshape(KT_D, 128).T).astype(f32)
    d["g2c"] = np.ascontiguousarray(ln2_g.reshape(KT_D, 128).T).astype(f32)
    d["be2c"] = np.ascontiguousarray(ln2_b.reshape(KT_D, 128).T).astype(f32)
    return d


def kernel(x, Wq, Wk, Wv, Wo, proj, W1, b1, W2, b2, ln1_g, ln1_b, ln2_g, ln2_b):
    from concourse import bass_utils

    x = np.asarray(x, np.float32)
    shared = _host_inputs(x, np.asarray(Wq), np.asarray(Wk), np.asarray(Wv),
                          np.asarray(Wo), np.asarray(proj), np.asarray(W1),
                          np.asarray(b1), np.asarray(W2), np.asarray(b2),
                          np.asarray(ln1_g), np.asarray(ln1_b),
                          np.asarray(ln2_g), np.asarray(ln2_b))

    if "nc" not in _CACHE:
        _CACHE["nc"] = _build()
    nc = _CACHE["nc"]

    in_maps = []
    for c in range(NC):
        xs = x[:, c * NT:(c + 1) * NT, :]
        xT = np.ascontiguousarray(xs.transpose(0, 2, 1))
        oh = np.zeros((1, NC), np.float32)
        oh[0, c] = 1.0
        m = dict(shared)
        m["x_f32"] = xT
        m["x_bf"] = xT.astype(ml_dtypes.bfloat16)
        m["onehot"] = oh
        in_maps.append(m)

    trace = bool(int(os.environ.get("KERNEL_TRACE", "0")))
    res = bass_utils.run_bass_kernel_spmd(nc, in_maps, core_ids=list(range(NC)),
                                          trace=trace)
    if trace and res.exec_time_ns is not None:
        print(f"HW exec time: {res.exec_time_ns} ns")
        if res.instructions_and_trace is not None:
            print("trace:", res.instructions_and_trace[1])

    outp = np.empty((B, N, D), np.float32)
    for c in range(NC):
        oT = res.results[c]["out"]
        outp[:, c * NT:(c + 1) * NT, :] = oT.transpose(0, 2, 1)
    return outp

